# revision 1
# baseline (speedup 1.0000x reference)
"""Trainium2 Bass kernel for DeepInstructedAttentionPositionScores.

Output [1, 8, 4116, 4116] f32 (~542 MB). Sharded one head per NeuronCore
(8 cores). Per core the kernel:
  - computes four tiny matvecs on PE (relative-position / cross scores),
  - expands them into Toeplitz base tiles via DRAM-scratch gather DMAs,
  - streams 32 stripes of [128, 4116]: one DVE broadcast-add each, then a
    ~2.1 MB contiguous store (alternating between the two HWDGE rings).
The kernel is HBM-write-bound: each core writes a contiguous 67.8 MB slab.
"""
import numpy as np

DIMQ = 4116
DIMI = 20
C_CONT = 0.125 / 3.0
C_CROSS = 0.125

_CACHE = {}
LAST_RESULTS = None


def _split_multi_waits(nc):
    """The walrus build in this image only encodes one semaphore wait per
    instruction. Tile emits multi-wait sync_info; split the extras into
    single-wait Drain instructions inserted just before, on the same engine
    (program order preserved, so semantics are unchanged)."""
    import concourse.mybir as mybir

    fn = nc.m.functions[0]
    ctr = 0
    for blk in fn.blocks:
        out = []
        for inst in blk.instructions:
            si = inst.sync_info
            waits = list(si.on_wait) if (si is not None and si.on_wait) else []
            if len(waits) > 1:
                for w in waits[:-1]:
                    ctr += 1
                    d = mybir.InstDrain(name=f"msw-{ctr}", ins=[], outs=[])
                    d.engine = inst.engine
                    d.sync_info = mybir.SyncInfo(on_wait=[w], on_update=[])
                    out.append(d)
                si.on_wait = waits[-1:]
            out.append(inst)
        blk.instructions = out
    return nc


def _build_nc(split=True):
    import concourse.bass as bass
    import concourse.mybir as mybir
    import concourse.tile as tile
    from contextlib import ExitStack

    DT = mybir.dt.float32
    nc = bass.Bass()
    tabw_d = nc.dram_tensor("tabw", [64, 213], DT, kind="ExternalInput")
    out_d = nc.dram_tensor("out", [DIMQ, DIMQ], DT, kind="ExternalOutput")
    scr_w = nc.dram_tensor("scr_w", [1, 63], DT)
    scr_d = nc.dram_tensor("scr_d", [1, 63], DT)
    scr_c = nc.dram_tensor("scr_c", [1, 20], DT)
    scr_h = nc.dram_tensor("scr_h", [31, 256], DT)

    with tile.TileContext(nc) as tc:
        with ExitStack() as ctx:
            const = ctx.enter_context(tc.tile_pool(name="const", bufs=1))
            psum = ctx.enter_context(tc.tile_pool(name="psum", bufs=2, space="PSUM"))
            opool = ctx.enter_context(tc.tile_pool(name="opool", bufs=1))

            tabw = const.tile([64, 213], DT)
            nc.sync.dma_start(tabw[:], tabw_d[:])

            # tiny matvecs on PE: hs as a [31,1] column, ws/ds/cs as rows
            p_hs = psum.tile([31, 1], DT, tag="pr")
            nc.tensor.matmul(p_hs[:], tabw[:, 16:47], tabw[:, 209:210])
            p_ws = psum.tile([1, 63], DT, tag="pr")
            nc.tensor.matmul(p_ws[:], tabw[:, 210:211], tabw[:, 63:126])
            p_ds = psum.tile([1, 63], DT, tag="pr")
            nc.tensor.matmul(p_ds[:], tabw[:, 211:212], tabw[:, 126:189])
            p_cs = psum.tile([1, 20], DT, tag="pr")
            nc.tensor.matmul(p_cs[:], tabw[:, 212:213], tabw[:, 189:209])

            hs_col = const.tile([31, 1], DT)
            nc.scalar.mul(hs_col[:], p_hs[:], C_CONT)
            ws_row = const.tile([1, 63], DT)
            nc.scalar.mul(ws_row[:], p_ws[:], C_CONT)
            ds_row = const.tile([1, 63], DT)
            nc.scalar.mul(ds_row[:], p_ds[:], C_CONT)
            cs_row = const.tile([1, 20], DT)
            nc.scalar.mul(cs_row[:], p_cs[:], C_CROSS)

            # hexp rows [31,256]: hs_col broadcast along free dim
            z31 = const.tile([31, 256], DT)
            nc.vector.memset(z31[:], 0.0)
            hx31 = const.tile([31, 256], DT)
            nc.vector.tensor_scalar_add(hx31[:], z31[:], hs_col[:])

            nc.sync.dma_start(scr_h[:], hx31[:])
            nc.sync.dma_start(scr_w[:], ws_row[:])
            nc.sync.dma_start(scr_d[:], ds_row[:])
            nc.sync.dma_start(scr_c[:], cs_row[:])

            # Big broadcast gathers FIRST on their queues so they are not
            # stuck behind the small window gathers; the high half of hexp is
            # what the first stripes read, so gather it first.
            hexp = const.tile([128, 7936], DT)
            nc.sync.dma_start(hexp[:, 3840:7936],
                              bass.AP(scr_h, 3840, [[0, 128], [1, 4096]]))
            nc.sync.dma_start(hexp[:, 0:3840],
                              bass.AP(scr_h, 0, [[0, 128], [1, 3840]]))
            crosst = const.tile([128, 20], DT)
            nc.scalar.dma_start(crosst[:], bass.AP(scr_c, 0, [[0, 128], [1, 20]]))

            # Small Toeplitz-window gathers, spread over the three
            # DMA-capable queues (sync/scalar/gpsimd) so they overlap.
            wcols = []
            for b in range(2):
                wcol = const.tile([128, 16], DT, tag=f"wcol{b}")
                for j1p in range(8):
                    j1 = 8 * b + j1p
                    eng = nc.scalar if b == 0 else nc.gpsimd
                    eng.dma_start(
                        wcol[16 * j1p:16 * (j1p + 1), :],
                        bass.AP(scr_w, 31 - j1, [[0, 16], [1, 16]]))
                wcols.append(wcol)
            dcol16 = const.tile([16, 16], DT)
            for k1 in range(16):
                eng = (nc.sync, nc.scalar, nc.gpsimd)[k1 % 3]
                eng.dma_start(dcol16[k1:k1 + 1, :],
                              bass.AP(scr_d, 31 - k1, [[1, 16]]))
            dcol = const.tile([128, 16], DT)
            for g in range(8):
                eng = (nc.sync, nc.scalar, nc.gpsimd)[g % 3]
                eng.dma_start(dcol[16 * g:16 * (g + 1), :], dcol16[:])

            # cB halves [128, 256], each built in ONE DVE op:
            # cb[p, 16*j2+k2] = dcol[p, k2] + wcol[p, j2]
            cb_reps = []
            da = dcol[:]
            d_rep = bass.AP(da.tensor, da.offset, [[16, 128], [0, 16], [1, 16]])
            for b in range(2):
                cb = const.tile([128, 256], DT, tag=f"cb{b}")
                wa = wcols[b][:]
                w_exp = bass.AP(wa.tensor, wa.offset, [[16, 128], [1, 16], [0, 16]])
                nc.vector.tensor_add(cb[:], d_rep, w_exp)
                ca = cb[:]
                cb_reps.append(bass.AP(ca.tensor, ca.offset,
                                       [[256, 128], [0, 16], [1, 256]]))

            # 8 persistent stripe tiles; cross cols [0:20] are written once,
            # then each stripe only rewrites the content cols and stores the
            # whole tile. Stripe k covers output rows 20+128k.
            slots = []
            for sidx in range(8):
                st = opool.tile([128, DIMQ], DT, tag=f"slot{sidx}")
                nc.vector.tensor_copy(st[:, 0:DIMI], crosst[:])
                slots.append(st)

            # zero rows 0..19
            zt = const.tile([128, DIMQ], DT)
            nc.gpsimd.memset(zt[:], 0.0)
            nc.gpsimd.dma_start(out_d[0:DIMI, :], zt[0:DIMI, :])
            for k in range(32):
                i1, b = k >> 1, k & 1
                ot = slots[k % 8]
                off = 256 * (15 - i1)
                nc.vector.tensor_add(ot[:, DIMI:DIMQ], cb_reps[b],
                                     hexp[:, off:off + 4096])
                r0 = DIMI + 128 * k
                eng = nc.sync if (k & 1) == 0 else nc.scalar
                eng.dma_start(out_d[r0:r0 + 128, :], ot[:])
    return _split_multi_waits(nc) if split else nc


def kernel(enc_cross, enc_h, enc_w, enc_d, w_cross, w_h, w_w, w_d,
           dim_q=4116, dim_k=4116, dim_i=20, dim_h=16, dim_w=16, dim_d=16,
           **_ignored):
    import os
    from concourse.bass_utils import run_bass_kernel_spmd
    global LAST_RESULTS

    enc_cross = np.asarray(enc_cross, dtype=np.float32)
    enc_h = np.asarray(enc_h, dtype=np.float32)
    enc_w = np.asarray(enc_w, dtype=np.float32)
    enc_d = np.asarray(enc_d, dtype=np.float32)
    w_cross = np.asarray(w_cross, dtype=np.float32)
    w_h = np.asarray(w_h, dtype=np.float32)
    w_w = np.asarray(w_w, dtype=np.float32)
    w_d = np.asarray(w_d, dtype=np.float32)

    cross2d = np.ascontiguousarray(enc_cross.reshape(DIMI, 64))
    tab = np.concatenate([enc_h.T, enc_w.T, enc_d.T, cross2d.T], axis=1)

    if "nc" not in _CACHE:
        _CACHE["nc"] = _build_nc()
    nc = _CACHE["nc"]

    core_ids = list(range(8))
    in_maps = []
    for h in core_ids:
        wp = np.stack([w_h[h], w_w[h], w_d[h], w_cross[h]], axis=1)
        tabw = np.ascontiguousarray(np.concatenate([tab, wp], axis=1))
        in_maps.append({"tabw": tabw})

    trace = bool(int(os.environ.get("KERNEL_TRACE", "0")))
    LAST_RESULTS = run_bass_kernel_spmd(nc, in_maps, core_ids, trace=trace)
    out = np.stack([LAST_RESULTS.results[h]["out"] for h in core_ids], axis=0)
    return out[None]

